# revision 3
# baseline (speedup 1.0000x reference)
"""Trainium2 Bass kernel for an 8-head self-attention block (MHA).

Problem: x[2, 4096, 512], 8 heads x 64 dims, torch-Linear q/k/v/o projections,
softmax attention, residual:  out = softmax(q k^T / 8) v @ Wo^T + bo + x.

Sharding (8 NeuronCores, no collectives): core c handles batch b = c // 4 and
query rows (c % 4) * 1024 ... + 1024, for ALL heads.  K/V for the full
sequence are computed on every core of a batch group, so the output
projection is fully local to a core.

Engine-level structure (from baseline trace analysis: ACT exp is a hard
~290us floor; PE was LDWEIGHTS-bound in PV and ran 195us at HAM half-clock):

  - heads processed in even/odd PAIRS: the K=64 score matmuls of the two
    heads live in disjoint PE row-groups (tile_position auto-derived from
    partition base 0 / 64) and run CONCURRENTLY in the 128x128 array.
  - PV in "transposed" orientation: stationary = [V | 1] chunk [s=128, 65]
    (65-column LDWEIGHTS, ~54ns), moving = P~ [s=128, q=512] -> psum
    oT[65, q] accumulated over all 32 s-chunks.  Row 64 collects the
    softmax denominator.  This kills the 2048 LDWEIGHTS x 107ns stream and
    all 64 PE transposes of the old orientation.
  - normalization: denominator row -> gpsimd partition_broadcast -> DVE
    fast reciprocal -> DVE multiply into bf16 oT (odd heads take a
    SBUF->SBUF DMA hop to partitions 64:128).
  - phase B (projections + pair01) uses q=512 score tiles so psum fits:
    pv01 accum 4 banks + {proj, scores} ring 4 banks.  Phase C (pairs
    23/45/67) uses q=1024 score tiles: sc ring 4 banks + pv 4 banks.
  - a dummy exp at t=0 preloads the ACT spline table during startup DMA.
"""

import numpy as np

B = 2
S = 4096
E = 512
H = 8
D = 64
P = 128
EC = E // P          # 4 e-chunks
FC = E // P          # 4 f-chunks
NJ = S // P          # 32 s-chunks
QR = S // 4          # 1024 query rows per core
NQS = QR // 512      # 2 query strips of 512
NKS = S // 512       # 8 s-strips of 512

_CACHE = {}


def _build_nc():
    import concourse.bass as bass
    import concourse.tile as tile
    from concourse import bacc, mybir

    f32 = mybir.dt.float32
    bf16 = mybir.dt.bfloat16
    AFT = mybir.ActivationFunctionType
    Alu = mybir.AluOpType

    nc = bacc.Bacc("TRN2", target_bir_lowering=False, debug=False, num_devices=8)

    xT_d = nc.declare_dram_parameter("xT", [E, S], bf16, isOutput=False)
    xqT_d = nc.declare_dram_parameter("xqT", [E, QR], bf16, isOutput=False)
    xres_d = nc.declare_dram_parameter("xres", [QR, E], f32, isOutput=False)
    wqT_d = nc.declare_dram_parameter("wqT", [E, E], bf16, isOutput=False)
    wkT_d = nc.declare_dram_parameter("wkT", [E, E], bf16, isOutput=False)
    wvT_d = nc.declare_dram_parameter("wvT", [E, E], bf16, isOutput=False)
    woT_d = nc.declare_dram_parameter("woT", [E, E], bf16, isOutput=False)
    bq_d = nc.declare_dram_parameter("bq", [P, FC], f32, isOutput=False)
    bk_d = nc.declare_dram_parameter("bk", [P, FC], f32, isOutput=False)
    bv_d = nc.declare_dram_parameter("bv", [E], f32, isOutput=False)
    out_d = nc.declare_dram_parameter("out", [QR, E], f32, isOutput=True)

    with tile.TileContext(nc) as tc:
        with tc.tile_pool(name="const", bufs=1) as const, \
             tc.tile_pool(name="persist", bufs=1) as persist:

            wo_sb = const.tile([P, EC, E], bf16)
            bq_sb = const.tile([P, FC], f32)
            bk_sb = const.tile([P, FC], f32)
            bv_sb = const.tile([P, E], f32)
            xres_sb = const.tile([P, QR // P, E], f32)

            kT_sb = persist.tile([P, FC, S], bf16)           # 32 KB/p
            qT_sb = persist.tile([P, FC, QR], bf16)          # 8 KB/p
            v_sb = persist.tile([P, NJ, H, 65], bf16)        # 32.5 KB/p
            oT_sb = persist.tile([P, FC, QR], bf16)          # 8 KB/p

            # constant-1 columns (softmax denominator lands in psum row 64)
            nc.vector.memset(v_sb[:, :, :, 64:65], 1.0)

            with tc.tile_pool(name="wpool", bufs=1) as wpool, \
                 tc.tile_pool(name="xtp", bufs=3) as xtp, \
                 tc.tile_pool(name="ptp", bufs=1) as ptp, \
                 tc.tile_pool(name="npool", bufs=1) as npool, \
                 tc.tile_pool(name="opool", bufs=2) as opool, \
                 tc.tile_pool(name="ps_sc", bufs=2, space="PSUM") as ps_sc, \
                 tc.tile_pool(name="ps_pv", bufs=2, space="PSUM") as ps_pv:

                # dummy exp: preloads the ACT table set during startup DMA
                dum = npool.tile([P, 1], f32, tag="dum", bufs=1)
                nc.vector.memset(dum[:], 0.0)
                nc.scalar.activation(dum[:], dum[:], AFT.Exp)

                wq_sb = wpool.tile([P, EC, E], bf16)
                wk_sb = wpool.tile([P, EC, E], bf16)
                wv_sb = wpool.tile([P, EC, E], bf16)
                # Q path first: its DMAs gate the first score matmul
                for e in range(EC):
                    nc.sync.dma_start(
                        out=wq_sb[:, e, :], in_=wqT_d[e * P:(e + 1) * P, :])
                nc.sync.dma_start(out=bq_sb[:], in_=bq_d[:])
                for t, d in ((wk_sb, wkT_d), (wv_sb, wvT_d)):
                    for e in range(EC):
                        nc.sync.dma_start(
                            out=t[:, e, :], in_=d[e * P:(e + 1) * P, :])
                nc.sync.dma_start(out=bk_sb[:], in_=bk_d[:])
                nc.sync.dma_start(
                    out=bv_sb[:],
                    in_=bass.AP(tensor=bv_d, offset=0, ap=[[0, P], [1, E]]))

                # ---- Q projection: qT[f, q] = (Wq @ xq^T + bq) / 8 ----
                for qs in range(NQS):
                    qsl = slice(qs * 512, (qs + 1) * 512)
                    xq = xtp.tile([P, EC, 512], bf16, tag="xt")
                    for e in range(EC):
                        nc.sync.dma_start(
                            out=xq[:, e, :], in_=xqT_d[e * P:(e + 1) * P, qsl])
                    for f in range(FC):
                        pq = ps_sc.tile([P, 512], f32, tag="sc", name="pq")
                        for e in range(EC):
                            nc.tensor.matmul(
                                pq[:], wq_sb[:, e, f * P:(f + 1) * P],
                                xq[:, e, :], start=(e == 0), stop=(e == EC - 1),
                                skip_group_check=True)
                        nc.vector.tensor_scalar(
                            qT_sb[:, f, qsl], pq[:], bq_sb[:, f:f + 1],
                            float(1.0 / np.sqrt(D)), Alu.add, Alu.mult)

                # ---- attention helpers ----

                def emit_pv(pair, j, pts, pvs, width):
                    # pts[(i, qs)] for width=512, pts[(i,)] sliced for 1024
                    for qs in range(2):
                        qsl = slice(qs * 512, (qs + 1) * 512)
                        for i, h in enumerate(pair):
                            if width == 512:
                                mv = pts[(i, qs)][:, :]
                            else:
                                mv = pts[(i,)][:, qsl]
                            nc.tensor.matmul(
                                pvs[i][0:65, qsl], v_sb[:, j, h, :], mv,
                                start=(j == 0), stop=(j == NJ - 1),
                                skip_group_check=True)

                def emit_normalize(pvs, pair):
                    # oT[0:64] = psum_rows_0:63 * (1 / psum_row_64), per head
                    for i, h in enumerate(pair):
                        fc = h // 2
                        fr = (h % 2) * 64
                        den = npool.tile([P, QR], f32, tag="den", bufs=2)
                        # cross-base DVE copy: psum partition 64 -> sbuf p0
                        # (partition_broadcast only honors a base-0 source)
                        nc.vector.tensor_copy(den[0:1, :], pvs[i][64:65, 0:QR])
                        stg = npool.tile([P, QR], f32, tag="stg", bufs=2)
                        nc.vector.tensor_copy(stg[0:64, :], pvs[i][0:64, 0:QR])
                        bc = npool.tile([P, QR], f32, tag="bc", bufs=2)
                        nc.gpsimd.partition_broadcast(
                            bc[0:64, :], den[0:1, :], channels=64)
                        rc = npool.tile([P, QR], f32, tag="rc", bufs=2)
                        nc.vector.reciprocal_approx_fast(rc[0:64, :], bc[0:64, :])
                        nc.vector.tensor_mul(
                            oT_sb[fr:fr + 64, fc, :], stg[0:64, :], rc[0:64, :])

                # ---- phase B: K/V projections interleaved with pair (0,1) ----
                pv01 = [ps_pv.tile([P, QR], f32, tag="pv", name=f"pv0{i}")
                        for i in range(2)]
                prev_pts = None
                for strip in range(NKS):
                    ssl = slice(strip * 512, (strip + 1) * 512)
                    xt = xtp.tile([P, EC, 512], bf16, tag="xt")
                    for e in range(EC):
                        nc.sync.dma_start(
                            out=xt[:, e, :], in_=xT_d[e * P:(e + 1) * P, ssl])
                    for f in range(FC):
                        pk = ps_sc.tile([P, 512], f32, tag="sc", name="pk")
                        for e in range(EC):
                            nc.tensor.matmul(
                                pk[:], wk_sb[:, e, f * P:(f + 1) * P],
                                xt[:, e, :], start=(e == 0), stop=(e == EC - 1),
                                skip_group_check=True)
                        nc.vector.tensor_scalar_add(
                            kT_sb[:, f, ssl], pk[:], bk_sb[:, f:f + 1])
                    for k in range(4):
                        j = strip * 4 + k
                        pvx = ps_sc.tile([P, E], f32, tag="sc", name="pvx")
                        for e in range(EC):
                            nc.tensor.matmul(
                                pvx[:], xt[:, e, k * P:(k + 1) * P],
                                wv_sb[:, e, :], start=(e == 0),
                                stop=(e == EC - 1), skip_group_check=True)
                        pv_v = pvx[:].rearrange("p (h d) -> p h d", h=H)
                        bv_v = bv_sb[:].rearrange("p (h d) -> p h d", h=H)
                        nc.vector.tensor_add(v_sb[:, j, :, 0:64], pv_v[:],
                                             bv_v[:])
                    # pair01 attention for this strip's 4 chunks, q in halves
                    for k in range(4):
                        j = strip * 4 + k
                        pts = {}
                        for qs in range(2):
                            qsl = slice(qs * 512, (qs + 1) * 512)
                            scs = []
                            for i in range(2):  # adjacent MMs -> row-tiled
                                fr = i * 64
                                sc = ps_sc.tile([P, 512], f32, tag="sc",
                                                name="scb")
                                nc.tensor.matmul(
                                    sc[:],
                                    kT_sb[fr:fr + 64, 0, j * P:(j + 1) * P],
                                    qT_sb[fr:fr + 64, 0, qsl],
                                    start=True, stop=True,
                                    skip_group_check=True)
                                scs.append(sc)
                            for i in range(2):
                                pt = ptp.tile([P, 512], bf16, tag="ptb",
                                              bufs=8)
                                nc.scalar.activation(pt[:], scs[i][:], AFT.Exp)
                                pts[(i, qs)] = pt
                        if prev_pts is not None:
                            emit_pv((0, 1), j - 1, prev_pts, pv01, 512)
                        prev_pts = pts
                emit_pv((0, 1), NJ - 1, prev_pts, pv01, 512)
                pending = (pv01, (0, 1))

                # tail-only data, off the startup critical path
                nc.sync.dma_start(
                    out=wo_sb[:],
                    in_=woT_d.ap().rearrange("(c p) f -> p c f", p=P))
                nc.sync.dma_start(
                    out=xres_sb[:],
                    in_=xres_d.ap().rearrange("(k p) f -> p k f", p=P))

                # ---- phase C: pairs (2,3), (4,5), (6,7), q=1024 tiles ----
                for pi in range(1, 4):
                    pair = (2 * pi, 2 * pi + 1)
                    fc = pi
                    pvs = [ps_pv.tile([P, QR], f32, tag="pv", name=f"pv{pi}{i}")
                           for i in range(2)]
                    prev_pts = None
                    for j in range(NJ):
                        jsl = slice(j * P, (j + 1) * P)
                        scs = []
                        for i in range(2):
                            scs.append(ps_sc.tile([P, QR], f32, tag="sc",
                                                  name="scc"))
                        for qs in range(2):
                            qsl = slice(qs * 512, (qs + 1) * 512)
                            for i in range(2):  # adjacent -> row-tiled pair
                                fr = i * 64
                                nc.tensor.matmul(
                                    scs[i][:, qsl],
                                    kT_sb[fr:fr + 64, fc, jsl],
                                    qT_sb[fr:fr + 64, fc, qsl],
                                    start=True, stop=True,
                                    skip_group_check=True)
                        pts = {}
                        for i in range(2):
                            pt = ptp.tile([P, QR], bf16, tag="ptc", bufs=5)
                            nc.scalar.activation(pt[:], scs[i][:], AFT.Exp)
                            pts[(i,)] = pt
                        if prev_pts is not None:
                            emit_pv(pair, j - 1, prev_pts, pvs, QR)
                        prev_pts = pts
                        if pending is not None and j == 1:
                            # previous pair's normalize runs inside this
                            # pair's stream on DVE/GpSimd while ACT chews exps
                            emit_normalize(*pending)
                            pending = None
                    emit_pv(pair, NJ - 1, prev_pts, pvs, QR)
                    pending = (pvs, pair)

                emit_normalize(*pending)

                # ---- output projection + residual ----
                for qc in range(QR // P):
                    po = ps_sc.tile([P, E], f32, tag="sc", name="po")
                    for e in range(EC):
                        nc.tensor.matmul(
                            po[:], oT_sb[:, e, qc * P:(qc + 1) * P],
                            wo_sb[:, e, :], start=(e == 0),
                            stop=(e == EC - 1), skip_group_check=True)
                    ot = opool.tile([P, E], f32, tag="ot", name="ot")
                    nc.vector.tensor_add(ot[:], po[:], xres_sb[:, qc, :])
                    nc.sync.dma_start(
                        out=out_d[qc * P:(qc + 1) * P, :], in_=ot[:])

    nc.compile()
    return nc


def _get_nc():
    if "nc" not in _CACHE:
        _CACHE["nc"] = _build_nc()
    return _CACHE["nc"]


def run_spmd(in_maps, **kw):
    from concourse.bass_utils import run_bass_kernel_spmd
    nc = _get_nc()
    return run_bass_kernel_spmd(nc, in_maps, list(range(8)), **kw)


def make_in_maps(x, Wq, bq, Wk, bk, Wv, bv, Wo, bo):
    import ml_dtypes
    bf = ml_dtypes.bfloat16
    x = np.asarray(x, dtype=np.float32)
    f32c = lambda a: np.ascontiguousarray(np.asarray(a, dtype=np.float32))
    bfc = lambda a: np.ascontiguousarray(
        np.asarray(a, dtype=np.float32).astype(bf))
    wqT = bfc(np.asarray(Wq).T)
    wkT = bfc(np.asarray(Wk).T)
    wvT = bfc(np.asarray(Wv).T)
    woT = bfc(np.asarray(Wo).T)
    bq_r = f32c(np.asarray(bq).reshape(FC, P).T)
    bk_r = f32c(np.asarray(bk).reshape(FC, P).T)
    bv_a = f32c(bv)
    bo_a = np.asarray(bo, dtype=np.float32)
    xT = [bfc(x[b].T) for b in range(B)]

    in_maps = []
    for c in range(8):
        b, r = c // 4, c % 4
        in_maps.append({
            "xT": xT[b],
            "xqT": np.ascontiguousarray(xT[b][:, r * QR:(r + 1) * QR]),
            # output bias folded into the residual tile (host-side, free)
            "xres": f32c(x[b, r * QR:(r + 1) * QR] + bo_a),
            "wqT": wqT, "wkT": wkT, "wvT": wvT, "woT": woT,
            "bq": bq_r, "bk": bk_r, "bv": bv_a,
        })
    return in_maps


def assemble(results):
    out = np.empty((B, S, E), dtype=np.float32)
    for c in range(8):
        b, r = c // 4, c % 4
        out[b, r * QR:(r + 1) * QR] = results[c]["out"]
    return out


def kernel(x, Wq, bq, Wk, bk, Wv, bv, Wo, bo):
    in_maps = make_in_maps(x, Wq, bq, Wk, bk, Wv, bv, Wo, bo)
    res = run_spmd(in_maps)
    return assemble(res.results)
